# revision 2
# baseline (speedup 1.0000x reference)
"""BayesianLinear Trainium2 kernel, 8-core SPMD (data-parallel over batch).

Per-core computation (4 samples each):
    w_b = weight_mean + noise_b * exp(0.5 * weight_logvar)   (B,O,I)
    out_b = x_b @ w_b^T + bias                               (B,L,O)

Design (per core, per sample):
  - std = exp(0.5*logvar) computed once on ACT, kept in natural (O-part) layout.
  - mean^T kept resident (PE-transposed once at startup).
  - noise chunk loaded natural, scaled by std on DVE (rounded to fp32r),
    PE-transposed (fp32r, exact permutation), fused mean-add evac on DVE -> w^T.
  - x loaded natural, PE-transposed (fp32), rounded on evac -> x^T.
  - out tile = x^T.T @ w^T via fp32r matmuls accumulated over 8 k-tiles,
    bias added via an extra K=1 matmul (ones^T @ bias), evac on ACT, store.
"""
import numpy as np

SAMPLES = 4           # batch samples per core
N_CORES = 8
B, L, I, O = 32, 512, 1024, 1024
KT = I // 128         # 8 k-tiles (contraction)
OT = O // 128         # 8 o-blocks
LT = L // 128         # 4 l-tiles
NCH = 4               # noise chunks per sample (2 o-blocks each)

_cache = {}


def _split_multi_waits(nc, mybir):
    """This walrus build allows at most one sync-wait per instruction; move
    extra waits onto preceding single-wait NOPs on the same engine.  Safe
    because kernel semaphores are monotonic between resets, so waiting
    sequentially is equivalent to waiting on the conjunction."""
    for fn in nc.m.functions:
        for bb in fn.blocks:
            insts = bb.instructions
            changed = False
            new_list = []
            for inst in insts:
                si = inst.sync_info
                if si is not None and si.on_wait and len(si.on_wait) > 1:
                    waits = list(si.on_wait)
                    for j, w in enumerate(waits[:-1]):
                        nop = mybir.InstNoOp(name=f"{inst.name}-w{j}", ins=[], outs=[])
                        nop.engine = inst.engine
                        nop.sync_info = mybir.SyncInfo(on_wait=[w], on_update=[])
                        new_list.append(nop)
                    inst.sync_info = mybir.SyncInfo(
                        on_wait=[waits[-1]], on_update=list(si.on_update or []))
                    changed = True
                new_list.append(inst)
            if changed:
                bb.instructions = new_list


def build_nc(use_f32r=True):
    from contextlib import ExitStack
    from concourse import bass, mybir, tile, masks

    F32 = mybir.dt.float32
    F32R = mybir.dt.float32r if use_f32r else mybir.dt.float32
    Exp = mybir.ActivationFunctionType.Exp
    Copy = mybir.ActivationFunctionType.Copy
    mult = mybir.AluOpType.mult
    add = mybir.AluOpType.add

    nc = bass.Bass()
    x_d = nc.declare_dram_parameter("x", [SAMPLES, L, I], F32, isOutput=False)
    nz_d = nc.declare_dram_parameter("noise", [SAMPLES, O, I], F32, isOutput=False)
    wm_d = nc.declare_dram_parameter("weight_mean", [O, I], F32, isOutput=False)
    wl_d = nc.declare_dram_parameter("weight_logvar", [O, I], F32, isOutput=False)
    b_d = nc.declare_dram_parameter("bias", [O], F32, isOutput=False)
    out_d = nc.declare_dram_parameter("out", [SAMPLES, L, O], F32, isOutput=True)

    with tile.TileContext(nc) as tc, ExitStack() as ctx:
        resident = ctx.enter_context(tc.tile_pool(name="resident", bufs=1))
        nat_pool = ctx.enter_context(tc.tile_pool(name="nat", bufs=3))
        scn_pool = ctx.enter_context(tc.tile_pool(name="scn", bufs=3))
        xnat_pool = ctx.enter_context(tc.tile_pool(name="xnat", bufs=1))
        wx_pool = ctx.enter_context(tc.tile_pool(name="wx", bufs=1))
        out_pool = ctx.enter_context(tc.tile_pool(name="outp", bufs=2))
        psum_mm = ctx.enter_context(tc.tile_pool(name="psum_mm", bufs=4, space="PSUM"))
        psum_nt = ctx.enter_context(tc.tile_pool(name="psum_nt", bufs=2, space="PSUM"))
        psum_xt = ctx.enter_context(tc.tile_pool(name="psum_xt", bufs=2, space="PSUM"))

        # ---------------- one-time setup ----------------
        std_nat = resident.tile([128, OT, I], F32, tag="std")    # exp(.5 lv), natural
        meanT = resident.tile([128, KT, O], F32, tag="meanT")    # mean^T
        ident = resident.tile([128, 128], F32, tag="ident")
        ident_r = resident.tile([128, 128], F32R, tag="ident_r")
        ones_f = resident.tile([1, 128], F32, tag="ones_f")
        ones_r = resident.tile([1, 128], F32R, tag="ones_r")
        bias_f = resident.tile([1, O], F32, tag="bias_f")
        bias_r = resident.tile([1, O], F32R, tag="bias_r")

        masks.make_identity(nc, ident[:])
        nc.vector.tensor_copy(ident_r[:], ident[:])
        nc.vector.memset(ones_f[:], 1.0)
        nc.vector.tensor_copy(ones_r[:], ones_f[:])
        nc.sync.dma_start(bias_f[:], b_d[:].rearrange("(a o) -> a o", a=1))
        nc.vector.tensor_copy(bias_r[:], bias_f[:])

        # logvar -> std (in place)
        nc.sync.dma_start(std_nat[:], wl_d[:].rearrange("(c p) i -> p c i", p=128))
        nc.scalar.activation(std_nat[:], std_nat[:], Exp, bias=0.0, scale=0.5)

        # mean -> mean^T via PE transposes (staged through the nat pool)
        for j in range(4):  # 4 slabs of 2 o-blocks
            mt = nat_pool.tile([128, 2, I], F32, tag="nat")
            nc.sync.dma_start(
                mt[:], wm_d[256 * j:256 * (j + 1), :].rearrange("(q p) i -> p q i", p=128))
            for q in range(2):
                ob = 2 * j + q
                for kh in range(2):  # k halves of 4
                    px = psum_xt.tile([128, 4, 128], F32, tag="pxt")
                    for kk in range(4):
                        k = 4 * kh + kk
                        nc.tensor.matmul(
                            px[:, kk, :], mt[:, q, 128 * k:128 * (k + 1)], ident[:],
                            is_transpose=True, start=True, stop=True)
                    nc.vector.tensor_copy(
                        meanT[:, 4 * kh:4 * (kh + 1), 128 * ob:128 * (ob + 1)], px[:])

        # ---------------- per-sample pipeline ----------------
        for b in range(SAMPLES):
            # w^T = mean^T + (noise*std)^T, built chunkwise
            wT = wx_pool.tile([128, KT, O], F32R, tag="wT")
            for c in range(NCH):
                nz = nat_pool.tile([128, 2, I], F32, tag="nat")
                nc.sync.dma_start(
                    nz[:], nz_d[b, 256 * c:256 * (c + 1), :].rearrange("(q p) i -> p q i", p=128))
                # scale by std in natural layout; round to fp32r
                sc = scn_pool.tile([128, 2, I], F32R, tag="scn")
                nc.vector.tensor_tensor(
                    sc[:], nz[:], std_nat[:, 2 * c:2 * (c + 1), :], mult)
                for q in range(2):
                    ob = 2 * c + q
                    for kh in range(2):
                        pn = psum_nt.tile([128, 4, 128], F32R, tag="pnt")
                        for kk in range(4):
                            k = 4 * kh + kk
                            nc.tensor.matmul(
                                pn[:, kk, :],
                                sc[:, q, 128 * k:128 * (k + 1)],
                                ident_r[:], is_transpose=True, start=True, stop=True)
                        # fused add of mean^T + evac (rounds to fp32r)
                        nc.vector.tensor_tensor(
                            wT[:, 4 * kh:4 * (kh + 1), 128 * ob:128 * (ob + 1)],
                            pn[:], meanT[:, 4 * kh:4 * (kh + 1), 128 * ob:128 * (ob + 1)],
                            add)

            # x^T
            xT = wx_pool.tile([128, KT, L], F32R, tag="xT")
            x_nat = xnat_pool.tile([128, LT, I], F32, tag="xnat")
            nc.sync.dma_start(x_nat[:], x_d[b].rearrange("(m p) i -> p m i", p=128))
            for m in range(LT):
                for kh in range(2):
                    px = psum_xt.tile([128, 4, 128], F32, tag="pxt")
                    for kk in range(4):
                        k = 4 * kh + kk
                        nc.tensor.matmul(
                            px[:, kk, :], x_nat[:, m, 128 * k:128 * (k + 1)], ident[:],
                            is_transpose=True, start=True, stop=True)
                    nc.vector.tensor_copy(
                        xT[:, 4 * kh:4 * (kh + 1), 128 * m:128 * (m + 1)], px[:])

            # matmuls: out[m] = x^T[:,k,m].T @ w^T[:,k,:] (+bias via K=1)
            for m in range(LT):
                pn0 = psum_mm.tile([128, 512], F32, tag="pmm")
                pn1 = psum_mm.tile([128, 512], F32, tag="pmm")
                for k in range(KT):
                    lhs = xT[:, k, 128 * m:128 * (m + 1)]
                    nc.tensor.matmul(pn0[:], lhs, wT[:, k, 0:512],
                                     start=(k == 0), stop=False)
                    nc.tensor.matmul(pn1[:], lhs, wT[:, k, 512:1024],
                                     start=(k == 0), stop=False)
                nc.tensor.matmul(pn0[:], ones_r[:], bias_r[:, 0:512],
                                 start=False, stop=True)
                nc.tensor.matmul(pn1[:], ones_r[:], bias_r[:, 512:1024],
                                 start=False, stop=True)
                out_t = out_pool.tile([128, O], F32, tag="out")
                nc.scalar.activation(out_t[:, 0:512], pn0[:], Copy)
                nc.scalar.activation(out_t[:, 512:1024], pn1[:], Copy)
                nc.scalar.dma_start(out_d[b, 128 * m:128 * (m + 1), :], out_t[:])

    _split_multi_waits(nc, mybir)
    return nc


def _get_nc(use_f32r=True):
    key = ("nc", use_f32r)
    if key not in _cache:
        _cache[key] = build_nc(use_f32r)
    return _cache[key]


def kernel(x, weight_mean, weight_logvar, bias, noise):
    from concourse import bass_utils

    x = np.ascontiguousarray(x, dtype=np.float32)
    noise = np.ascontiguousarray(noise, dtype=np.float32)
    weight_mean = np.ascontiguousarray(weight_mean, dtype=np.float32)
    weight_logvar = np.ascontiguousarray(weight_logvar, dtype=np.float32)
    bias = np.ascontiguousarray(bias, dtype=np.float32)

    nc = _get_nc()
    in_maps = []
    for c in range(N_CORES):
        sl = slice(SAMPLES * c, SAMPLES * (c + 1))
        in_maps.append({
            "x": x[sl], "noise": noise[sl],
            "weight_mean": weight_mean, "weight_logvar": weight_logvar,
            "bias": bias,
        })
    res = bass_utils.run_bass_kernel_spmd(nc, in_maps, list(range(N_CORES)))
    out = np.concatenate([res.results[c]["out"] for c in range(N_CORES)], axis=0)
    return out.astype(np.float32)


if __name__ == "__main__":
    rng = np.random.default_rng(0)
    xs = rng.standard_normal((B, L, I), dtype=np.float32)
    wm = (rng.standard_normal((O, I)) * 0.02).astype(np.float32)
    wl = (-rng.random((O, I)) * 6.0 - 2.0).astype(np.float32)
    bs = (rng.standard_normal(O) * 0.02).astype(np.float32)
    nz = rng.standard_normal((B, O, I), dtype=np.float32)
    got = kernel(x=xs, weight_mean=wm, weight_logvar=wl, bias=bs, noise=nz)
    w = wm + nz * np.exp(0.5 * wl)
    want = np.einsum("bli,boi->blo", xs.astype(np.float64), w.astype(np.float64)) + bs
    err = np.abs(got - want).max() / np.abs(want).max()
    print("relerr:", err)


# revision 3
# speedup vs baseline: 1.0628x; 1.0628x over previous
"""BayesianLinear Trainium2 kernel, 8-core SPMD (data-parallel over batch).

Per-core computation (4 samples each):
    w_b = weight_mean + noise_b * exp(0.5 * weight_logvar)   (B,O,I)
    out_b = x_b @ w_b^T + bias                               (B,L,O)

Design (per core):
  - std = exp(0.5*logvar) once on ACT, kept natural (O on partitions).
  - mean^T resident (PE-transposed once at startup).
  - Per sample, software-pipelined in column halves so PE stays dense:
      [x load + PE-transpose x (ACT rounds to fp32r on evac)]
      [noise chunks 0,1: DVE/GpSimd scale-mul (fp32r round), PE transpose,
       DVE fused mean-add evac -> w^T cols 0-511]
      [matmuls n=0: psum += x^T.T @ w^T over 8 k-tiles, K=1 bias matmul,
       ACT evac, store]
      [chunks 2,3 -> w^T cols 512-1023]  [matmuls n=1]
  - fp32r matmuls run the PE at bf16 rate (1 cyc/row); fp32r transposes are
    exact permutations; rounding (~2^-12) happens once per operand.
"""
import numpy as np

SAMPLES = 4           # batch samples per core
N_CORES = 8
B, L, I, O = 32, 512, 1024, 1024
KT = I // 128         # 8 k-tiles (contraction)
OT = O // 128         # 8 o-blocks
LT = L // 128         # 4 l-tiles
NCH = 4               # noise chunks per sample (2 o-blocks each)

_cache = {}


def _split_multi_waits(nc, mybir):
    """This walrus build allows at most one sync-wait per instruction; move
    extra waits onto preceding single-wait NOPs on the same engine.  Safe
    because kernel semaphores are monotonic between resets, so waiting
    sequentially is equivalent to waiting on the conjunction."""
    for fn in nc.m.functions:
        for bb in fn.blocks:
            insts = bb.instructions
            changed = False
            new_list = []
            for inst in insts:
                si = inst.sync_info
                if si is not None and si.on_wait and len(si.on_wait) > 1:
                    waits = list(si.on_wait)
                    for j, w in enumerate(waits[:-1]):
                        nop = mybir.InstNoOp(name=f"{inst.name}-w{j}", ins=[], outs=[])
                        nop.engine = inst.engine
                        nop.sync_info = mybir.SyncInfo(on_wait=[w], on_update=[])
                        new_list.append(nop)
                    inst.sync_info = mybir.SyncInfo(
                        on_wait=[waits[-1]], on_update=list(si.on_update or []))
                    changed = True
                new_list.append(inst)
            if changed:
                bb.instructions = new_list


def build_nc(use_f32r=True):
    from contextlib import ExitStack
    from concourse import bass, mybir, tile, masks

    F32 = mybir.dt.float32
    F32R = mybir.dt.float32r if use_f32r else mybir.dt.float32
    Exp = mybir.ActivationFunctionType.Exp
    Copy = mybir.ActivationFunctionType.Copy
    mult = mybir.AluOpType.mult
    add = mybir.AluOpType.add

    nc = bass.Bass()
    x_d = nc.declare_dram_parameter("x", [SAMPLES, L, I], F32, isOutput=False)
    nz_d = nc.declare_dram_parameter("noise", [SAMPLES, O, I], F32, isOutput=False)
    wm_d = nc.declare_dram_parameter("weight_mean", [O, I], F32, isOutput=False)
    wl_d = nc.declare_dram_parameter("weight_logvar", [O, I], F32, isOutput=False)
    b_d = nc.declare_dram_parameter("bias", [O], F32, isOutput=False)
    out_d = nc.declare_dram_parameter("out", [SAMPLES, L, O], F32, isOutput=True)

    with tile.TileContext(nc) as tc, ExitStack() as ctx:
        resident = ctx.enter_context(tc.tile_pool(name="resident", bufs=1))
        nat_pool = ctx.enter_context(tc.tile_pool(name="nat", bufs=3))
        scn_pool = ctx.enter_context(tc.tile_pool(name="scn", bufs=2))
        xnat_pool = ctx.enter_context(tc.tile_pool(name="xnat", bufs=2))
        wx_pool = ctx.enter_context(tc.tile_pool(name="wx", bufs=1))
        out_pool = ctx.enter_context(tc.tile_pool(name="outp", bufs=3))
        psum_mm = ctx.enter_context(tc.tile_pool(name="psum_mm", bufs=3, space="PSUM"))
        psum_nt = ctx.enter_context(tc.tile_pool(name="psum_nt", bufs=3, space="PSUM"))
        psum_xt = ctx.enter_context(tc.tile_pool(name="psum_xt", bufs=2, space="PSUM"))

        # ---------------- one-time setup ----------------
        std_nat = resident.tile([128, OT, I], F32, tag="std")    # exp(.5 lv), natural
        meanT = resident.tile([128, KT, O], F32, tag="meanT")    # mean^T
        ident = resident.tile([128, 128], F32, tag="ident")
        ident_r = resident.tile([128, 128], F32R, tag="ident_r")
        ones_f = resident.tile([1, 128], F32, tag="ones_f")
        ones_r = resident.tile([1, 128], F32R, tag="ones_r")
        bias_f = resident.tile([1, O], F32, tag="bias_f")
        bias_r = resident.tile([1, O], F32R, tag="bias_r")

        masks.make_identity(nc, ident[:])
        nc.vector.tensor_copy(ident_r[:], ident[:])
        nc.vector.memset(ones_f[:], 1.0)
        nc.vector.tensor_copy(ones_r[:], ones_f[:])
        nc.sync.dma_start(bias_f[:], b_d[:].rearrange("(a o) -> a o", a=1))
        nc.vector.tensor_copy(bias_r[:], bias_f[:])

        # logvar -> std (in place)
        nc.sync.dma_start(std_nat[:], wl_d[:].rearrange("(c p) i -> p c i", p=128))
        nc.scalar.activation(std_nat[:], std_nat[:], Exp, bias=0.0, scale=0.5)

        # mean -> mean^T via PE transposes (staged through the nat pool)
        for j in range(4):  # 4 slabs of 2 o-blocks
            mt = nat_pool.tile([128, 2, I], F32, tag="nat")
            nc.sync.dma_start(
                mt[:], wm_d[256 * j:256 * (j + 1), :].rearrange("(q p) i -> p q i", p=128))
            for q in range(2):
                ob = 2 * j + q
                for kh in range(2):  # k halves of 4
                    px = psum_xt.tile([128, 4, 128], F32, tag="pxt")
                    for kk in range(4):
                        k = 4 * kh + kk
                        nc.tensor.matmul(
                            px[:, kk, :], mt[:, q, 128 * k:128 * (k + 1)], ident[:],
                            is_transpose=True, start=True, stop=True)
                    nc.vector.tensor_copy(
                        meanT[:, 4 * kh:4 * (kh + 1), 128 * ob:128 * (ob + 1)], px[:])

        # ---------------- per-sample pipeline ----------------
        def emit_chunk(b, c, wT):
            """noise chunk c (o-blocks 2c, 2c+1): load, scale, transpose, add."""
            nz = nat_pool.tile([128, 2, I], F32, tag="nat")
            nc.sync.dma_start(
                nz[:], nz_d[b, 256 * c:256 * (c + 1), :].rearrange("(q p) i -> p q i", p=128))
            sc = scn_pool.tile([128, 2, I], F32R, tag="scn")
            # alternate scale-muls between DVE and GpSimd
            eng = nc.vector if c % 2 == 0 else nc.gpsimd
            eng.tensor_tensor(sc[:], nz[:], std_nat[:, 2 * c:2 * (c + 1), :], mult)
            for q in range(2):
                ob = 2 * c + q
                for kh in range(2):
                    pn = psum_nt.tile([128, 4, 128], F32R, tag="pnt")
                    for kk in range(4):
                        k = 4 * kh + kk
                        nc.tensor.matmul(
                            pn[:, kk, :], sc[:, q, 128 * k:128 * (k + 1)],
                            ident_r[:], is_transpose=True, start=True, stop=True)
                    nc.vector.tensor_tensor(
                        wT[:, 4 * kh:4 * (kh + 1), 128 * ob:128 * (ob + 1)],
                        pn[:], meanT[:, 4 * kh:4 * (kh + 1), 128 * ob:128 * (ob + 1)],
                        add)

        def emit_mm_half(b, n, wT, xT):
            """matmuls for output columns [512n, 512(n+1))."""
            for m in range(LT):
                pm = psum_mm.tile([128, 512], F32, tag="pmm")
                for k in range(KT):
                    nc.tensor.matmul(pm[:], xT[:, k, 128 * m:128 * (m + 1)],
                                     wT[:, k, 512 * n:512 * (n + 1)],
                                     start=(k == 0), stop=False)
                nc.tensor.matmul(pm[:], ones_r[:], bias_r[:, 512 * n:512 * (n + 1)],
                                 start=False, stop=True)
                ot = out_pool.tile([128, 512], F32, tag="out")
                nc.scalar.activation(ot[:], pm[:], Copy)
                nc.scalar.dma_start(
                    out_d[b, 128 * m:128 * (m + 1), 512 * n:512 * (n + 1)], ot[:])

        for b in range(SAMPLES):
            # x^T build (ACT rounds on evac)
            xT = wx_pool.tile([128, KT, L], F32R, tag="xT")
            x_nat = xnat_pool.tile([128, LT, I], F32, tag="xnat")
            nc.sync.dma_start(x_nat[:], x_d[b].rearrange("(m p) i -> p m i", p=128))
            for m in range(LT):
                for kh in range(2):
                    px = psum_xt.tile([128, 4, 128], F32, tag="pxt")
                    for kk in range(4):
                        k = 4 * kh + kk
                        nc.tensor.matmul(
                            px[:, kk, :], x_nat[:, m, 128 * k:128 * (k + 1)], ident[:],
                            is_transpose=True, start=True, stop=True)
                    nc.scalar.activation(
                        xT[:, 4 * kh:4 * (kh + 1), 128 * m:128 * (m + 1)], px[:], Copy)

            wT = wx_pool.tile([128, KT, O], F32R, tag="wT")
            for half in range(2):
                emit_chunk(b, 2 * half, wT)
                emit_chunk(b, 2 * half + 1, wT)
                emit_mm_half(b, half, wT, xT)

    _split_multi_waits(nc, mybir)
    return nc


def _get_nc(use_f32r=True):
    key = ("nc", use_f32r)
    if key not in _cache:
        _cache[key] = build_nc(use_f32r)
    return _cache[key]


def kernel(x, weight_mean, weight_logvar, bias, noise):
    from concourse import bass_utils

    x = np.ascontiguousarray(x, dtype=np.float32)
    noise = np.ascontiguousarray(noise, dtype=np.float32)
    weight_mean = np.ascontiguousarray(weight_mean, dtype=np.float32)
    weight_logvar = np.ascontiguousarray(weight_logvar, dtype=np.float32)
    bias = np.ascontiguousarray(bias, dtype=np.float32)

    nc = _get_nc()
    in_maps = []
    for c in range(N_CORES):
        sl = slice(SAMPLES * c, SAMPLES * (c + 1))
        in_maps.append({
            "x": x[sl], "noise": noise[sl],
            "weight_mean": weight_mean, "weight_logvar": weight_logvar,
            "bias": bias,
        })
    res = bass_utils.run_bass_kernel_spmd(nc, in_maps, list(range(N_CORES)))
    out = np.concatenate([res.results[c]["out"] for c in range(N_CORES)], axis=0)
    return out.astype(np.float32)


# revision 4
# speedup vs baseline: 1.0630x; 1.0001x over previous
"""BayesianLinear Trainium2 kernel, 8-core SPMD (data-parallel over batch).

Per-core computation (4 samples each):
    w_b = weight_mean + noise_b * exp(0.5 * weight_logvar)   (B,O,I)
    out_b = x_b @ w_b^T + bias                               (B,L,O)

Design (per core):
  - std = exp(0.5*logvar) once on ACT, kept natural (O on partitions).
  - mean^T resident (PE-transposed once at startup).
  - Per sample, software-pipelined in column halves so PE stays dense:
      [x load + PE-transpose x (ACT rounds to fp32r on evac)]
      [noise chunks 0,1: DVE/GpSimd scale-mul (fp32r round), PE transpose,
       DVE fused mean-add evac -> w^T cols 0-511]
      [matmuls n=0: psum += x^T.T @ w^T over 8 k-tiles, K=1 bias matmul,
       ACT evac, store]
      [chunks 2,3 -> w^T cols 512-1023]  [matmuls n=1]
  - fp32r matmuls run the PE at bf16 rate (1 cyc/row); fp32r transposes are
    exact permutations; rounding (~2^-12) happens once per operand.
"""
import numpy as np

SAMPLES = 4           # batch samples per core
N_CORES = 8
B, L, I, O = 32, 512, 1024, 1024
KT = I // 128         # 8 k-tiles (contraction)
OT = O // 128         # 8 o-blocks
LT = L // 128         # 4 l-tiles
NCH = 4               # noise chunks per sample (2 o-blocks each)

_cache = {}


def _split_multi_waits(nc, mybir):
    """This walrus build allows at most one sync-wait per instruction; move
    extra waits onto preceding single-wait NOPs on the same engine.  Safe
    because kernel semaphores are monotonic between resets, so waiting
    sequentially is equivalent to waiting on the conjunction."""
    for fn in nc.m.functions:
        for bb in fn.blocks:
            insts = bb.instructions
            changed = False
            new_list = []
            for inst in insts:
                si = inst.sync_info
                if si is not None and si.on_wait and len(si.on_wait) > 1:
                    waits = list(si.on_wait)
                    for j, w in enumerate(waits[:-1]):
                        nop = mybir.InstNoOp(name=f"{inst.name}-w{j}", ins=[], outs=[])
                        nop.engine = inst.engine
                        nop.sync_info = mybir.SyncInfo(on_wait=[w], on_update=[])
                        new_list.append(nop)
                    inst.sync_info = mybir.SyncInfo(
                        on_wait=[waits[-1]], on_update=list(si.on_update or []))
                    changed = True
                new_list.append(inst)
            if changed:
                bb.instructions = new_list


def build_nc(use_f32r=True):
    from contextlib import ExitStack
    from concourse import bass, mybir, tile, masks

    F32 = mybir.dt.float32
    F32R = mybir.dt.float32r if use_f32r else mybir.dt.float32
    Exp = mybir.ActivationFunctionType.Exp
    Copy = mybir.ActivationFunctionType.Copy
    mult = mybir.AluOpType.mult
    add = mybir.AluOpType.add

    nc = bass.Bass()
    x_d = nc.declare_dram_parameter("x", [SAMPLES, L, I], F32, isOutput=False)
    nz_d = nc.declare_dram_parameter("noise", [SAMPLES, O, I], F32, isOutput=False)
    wm_d = nc.declare_dram_parameter("weight_mean", [O, I], F32, isOutput=False)
    wl_d = nc.declare_dram_parameter("weight_logvar", [O, I], F32, isOutput=False)
    b_d = nc.declare_dram_parameter("bias", [O], F32, isOutput=False)
    out_d = nc.declare_dram_parameter("out", [SAMPLES, L, O], F32, isOutput=True)

    with tile.TileContext(nc) as tc, ExitStack() as ctx:
        resident = ctx.enter_context(tc.tile_pool(name="resident", bufs=1))
        nat_pool = ctx.enter_context(tc.tile_pool(name="nat", bufs=3))
        scn_pool = ctx.enter_context(tc.tile_pool(name="scn", bufs=2))
        xnat_pool = ctx.enter_context(tc.tile_pool(name="xnat", bufs=2))
        wx_pool = ctx.enter_context(tc.tile_pool(name="wx", bufs=1))
        out_pool = ctx.enter_context(tc.tile_pool(name="outp", bufs=3))
        psum_mm = ctx.enter_context(tc.tile_pool(name="psum_mm", bufs=3, space="PSUM"))
        psum_nt = ctx.enter_context(tc.tile_pool(name="psum_nt", bufs=3, space="PSUM"))
        psum_xt = ctx.enter_context(tc.tile_pool(name="psum_xt", bufs=2, space="PSUM"))

        # ---------------- one-time setup ----------------
        std_nat = resident.tile([128, OT, I], F32, tag="std")    # exp(.5 lv), natural
        meanT = resident.tile([128, KT, O], F32, tag="meanT")    # mean^T
        ident = resident.tile([128, 128], F32, tag="ident")
        ident_r = resident.tile([128, 128], F32R, tag="ident_r")
        ones_f = resident.tile([1, 128], F32, tag="ones_f")
        ones_r = resident.tile([1, 128], F32R, tag="ones_r")
        bias_f = resident.tile([1, O], F32, tag="bias_f")
        bias_r = resident.tile([1, O], F32R, tag="bias_r")

        masks.make_identity(nc, ident[:])
        nc.vector.tensor_copy(ident_r[:], ident[:])
        nc.vector.memset(ones_f[:], 1.0)
        nc.vector.tensor_copy(ones_r[:], ones_f[:])
        nc.sync.dma_start(bias_f[:], b_d[:].rearrange("(a o) -> a o", a=1))
        nc.vector.tensor_copy(bias_r[:], bias_f[:])

        # PE pre-warm: dummy transposes keep the HAM activity monitor busy so
        # the 2.4 GHz clock is ungated by the time real work arrives.
        warm = resident.tile([128, 128], F32, tag="warm")
        nc.vector.memset(warm[:], 1.0)
        pwarm = psum_xt.tile([128, 4, 128], F32, tag="pxt")
        for _ in range(48):
            nc.tensor.matmul(pwarm[:, 0, :], warm[:], ident[:],
                             is_transpose=True, start=True, stop=True)

        # mean -> mean^T via PE transposes; logvar -> std per slab, interleaved
        # so compute starts as soon as the first slab lands.
        for j in range(4):  # 4 slabs of 2 o-blocks
            sl = slice(256 * j, 256 * (j + 1))
            mt = nat_pool.tile([128, 2, I], F32, tag="nat")
            nc.sync.dma_start(
                mt[:], wm_d[sl, :].rearrange("(q p) i -> p q i", p=128))
            nc.sync.dma_start(std_nat[:, 2 * j:2 * (j + 1), :],
                              wl_d[sl, :].rearrange("(q p) i -> p q i", p=128))
            nc.scalar.activation(std_nat[:, 2 * j:2 * (j + 1), :],
                                 std_nat[:, 2 * j:2 * (j + 1), :],
                                 Exp, bias=0.0, scale=0.5)
            for q in range(2):
                ob = 2 * j + q
                for kh in range(2):  # k halves of 4
                    px = psum_xt.tile([128, 4, 128], F32, tag="pxt")
                    for kk in range(4):
                        k = 4 * kh + kk
                        nc.tensor.matmul(
                            px[:, kk, :], mt[:, q, 128 * k:128 * (k + 1)], ident[:],
                            is_transpose=True, start=True, stop=True)
                    nc.vector.tensor_copy(
                        meanT[:, 4 * kh:4 * (kh + 1), 128 * ob:128 * (ob + 1)], px[:])

        # ---------------- per-sample pipeline ----------------
        def emit_chunk(b, c, wT):
            """noise chunk c (o-blocks 2c, 2c+1): load, scale, transpose, add."""
            nz = nat_pool.tile([128, 2, I], F32, tag="nat")
            nc.sync.dma_start(
                nz[:], nz_d[b, 256 * c:256 * (c + 1), :].rearrange("(q p) i -> p q i", p=128))
            sc = scn_pool.tile([128, 2, I], F32R, tag="scn")
            # alternate scale-muls between DVE and GpSimd
            eng = nc.gpsimd if c == 3 else nc.vector
            eng.tensor_tensor(sc[:], nz[:], std_nat[:, 2 * c:2 * (c + 1), :], mult)
            for q in range(2):
                ob = 2 * c + q
                for kh in range(2):
                    pn = psum_nt.tile([128, 4, 128], F32R, tag="pnt")
                    for kk in range(4):
                        k = 4 * kh + kk
                        nc.tensor.matmul(
                            pn[:, kk, :], sc[:, q, 128 * k:128 * (k + 1)],
                            ident_r[:], is_transpose=True, start=True, stop=True)
                    nc.vector.tensor_tensor(
                        wT[:, 4 * kh:4 * (kh + 1), 128 * ob:128 * (ob + 1)],
                        pn[:], meanT[:, 4 * kh:4 * (kh + 1), 128 * ob:128 * (ob + 1)],
                        add)

        def emit_mm_half(b, n, wT, xT):
            """matmuls for output columns [512n, 512(n+1))."""
            for m in range(LT):
                pm = psum_mm.tile([128, 512], F32, tag="pmm")
                for k in range(KT):
                    nc.tensor.matmul(pm[:], xT[:, k, 128 * m:128 * (m + 1)],
                                     wT[:, k, 512 * n:512 * (n + 1)],
                                     start=(k == 0), stop=False)
                nc.tensor.matmul(pm[:], ones_r[:], bias_r[:, 512 * n:512 * (n + 1)],
                                 start=False, stop=True)
                ot = out_pool.tile([128, 512], F32, tag="out")
                nc.scalar.activation(ot[:], pm[:], Copy)
                nc.scalar.dma_start(
                    out_d[b, 128 * m:128 * (m + 1), 512 * n:512 * (n + 1)], ot[:])

        for b in range(SAMPLES):
            # x^T build (ACT rounds on evac)
            xT = wx_pool.tile([128, KT, L], F32R, tag="xT")
            x_nat = xnat_pool.tile([128, LT, I], F32, tag="xnat")
            nc.sync.dma_start(x_nat[:], x_d[b].rearrange("(m p) i -> p m i", p=128))
            for m in range(LT):
                for kh in range(2):
                    px = psum_xt.tile([128, 4, 128], F32, tag="pxt")
                    for kk in range(4):
                        k = 4 * kh + kk
                        nc.tensor.matmul(
                            px[:, kk, :], x_nat[:, m, 128 * k:128 * (k + 1)], ident[:],
                            is_transpose=True, start=True, stop=True)
                    nc.scalar.activation(
                        xT[:, 4 * kh:4 * (kh + 1), 128 * m:128 * (m + 1)], px[:], Copy)

            wT = wx_pool.tile([128, KT, O], F32R, tag="wT")
            for half in range(2):
                emit_chunk(b, 2 * half, wT)
                emit_chunk(b, 2 * half + 1, wT)
                emit_mm_half(b, half, wT, xT)

    _split_multi_waits(nc, mybir)
    return nc


def _get_nc(use_f32r=True):
    key = ("nc", use_f32r)
    if key not in _cache:
        _cache[key] = build_nc(use_f32r)
    return _cache[key]


def kernel(x, weight_mean, weight_logvar, bias, noise):
    from concourse import bass_utils

    x = np.ascontiguousarray(x, dtype=np.float32)
    noise = np.ascontiguousarray(noise, dtype=np.float32)
    weight_mean = np.ascontiguousarray(weight_mean, dtype=np.float32)
    weight_logvar = np.ascontiguousarray(weight_logvar, dtype=np.float32)
    bias = np.ascontiguousarray(bias, dtype=np.float32)

    nc = _get_nc()
    in_maps = []
    for c in range(N_CORES):
        sl = slice(SAMPLES * c, SAMPLES * (c + 1))
        in_maps.append({
            "x": x[sl], "noise": noise[sl],
            "weight_mean": weight_mean, "weight_logvar": weight_logvar,
            "bias": bias,
        })
    res = bass_utils.run_bass_kernel_spmd(nc, in_maps, list(range(N_CORES)))
    out = np.concatenate([res.results[c]["out"] for c in range(N_CORES)], axis=0)
    return out.astype(np.float32)
